# revision 20
# baseline (speedup 1.0000x reference)
"""Trainium2 Bass kernel for nn_Attention_8839042695176.

Full (unsharded) inputs in, full output out. Internally: 8 NeuronCores,
core h owns attention head h (both batch elements).

Math per (b, h) unit:
    scores[i,j] = q_full[c,i]·emb[c,j] + qd_up[c,i]·kd_up[c,j]   (K=16 matmul)
    attn = softmax_j(scores)        (no max-subtraction; |scores| <~ 8)
    out[c,i]  = sum_j attn[i,j] vv[c,j]
computed in transposed layout E^T[j,i] so both big matmuls stream on PE,
with the softmax denominator fused in as an extra all-ones row of vv^T.

v2 prologue: the 1x1 qkv conv is folded into the 11x11 strided conv on
the host (wq' = wq @ W1q over input channels) so the conv reads a
host-prepadded, pre-shifted copy of f (XF) straight from HBM — no
on-device padded-map construction. Only each head's 8 output channels
of the conv are computed. v^T tiles come from direct transposed matmuls
on f. Staging is spread across DVE/ACT/Pool so batch 1's staging
overlaps batch 0's ACT-bound main loop.
"""

import numpy as np

HEADS = 8
DIM_HEAD = 8
B = 2
C = 64
H = 48
HW = H * H          # 2304
KS = 11             # conv kernel
STRIDE = 8
PAD = 2
M6 = 6              # downsampled side
MM = M6 * M6        # 36
PADW = H + 2 * PAD  # 52
PADHW = PADW * PADW # 2704
SCALE = DIM_HEAD ** (-0.5)
NCORES = 8
NSLOT = 66          # conv tap slots (5 kx-pairs + 1 single per ky row)

# i-chunks for the main loop (<=512 fp32 moving limit); 256 last so the
# final drain chain (rec/broadcast/out-DMA) is short.
ICHUNKS = [(0, 512), (512, 512), (1024, 512), (1536, 512), (2048, 256)]
NJT = HW // 128     # 18 j-tiles

_PROGRAMS = {}
# Conv activation selector: "Gelu" on hardware; CoreSim lacks Gelu, so the
# sim test swaps in "Tanh" (np reference adapted identically).
_CONV_ACT = "Gelu"


def _build_program(repeat=1, split=True):
    from contextlib import ExitStack
    import concourse.bass as bass
    import concourse.mybir as mybir
    import concourse.tile as tile

    F32 = mybir.dt.float32
    BF = mybir.dt.bfloat16
    AF = mybir.ActivationFunctionType

    nc = bass.Bass(trn_type="TRN2")

    xf = nc.declare_dram_parameter("xf", [128, B, PADHW], BF, isOutput=False)
    f2 = nc.declare_dram_parameter("f2", [128, HW], BF, isOutput=False)
    w1qT = nc.declare_dram_parameter("w1qT", [128, 8], BF, isOutput=False)
    wvT = nc.declare_dram_parameter("wvT", [128, 8], BF, isOutput=False)
    wcT = nc.declare_dram_parameter("wcT", [128, NSLOT * 40], BF, isOutput=False)
    bqk = nc.declare_dram_parameter("bqk", [40], F32, isOutput=False)
    emb = nc.declare_dram_parameter("emb", [8, HW], BF, isOutput=False)
    out = nc.declare_dram_parameter("out", [B, 8, HW], F32, isOutput=True)

    def up_ap(Dt, b, krows=False):
        """Broadcast view: D[c, b, p] -> [c, 36(p), 64(repeat)] (flat i//64).
        krows selects partition rows 32-39 (the k half of the QK tile;
        k sits at 32 because partition bases must be multiples of 32)."""
        return bass.AP(
            tensor=Dt.tensor,
            offset=Dt.offset + b * MM + (32 * Dt.ap[0][0] if krows else 0),
            ap=[[Dt.ap[0][0], 8], [1, MM], [0, 64]],
        )

    with tile.TileContext(nc) as tc, ExitStack() as ctx:
        # persistent pools (whole kernel)
        const = ctx.enter_context(tc.tile_pool(name="const", bufs=1))
        work = ctx.enter_context(tc.tile_pool(name="work", bufs=3))
        epool = ctx.enter_context(tc.tile_pool(name="epool", bufs=3))

        ONE9 = const.tile([1, 9], F32)
        nc.vector.memset(ONE9, 1.0)

        def _rep_body():
            # ---- persistent tiles ----
            XF = const.tile([128, B, PADHW], BF, name="XF")
            F2 = const.tile([128, HW], BF, name="F2")
            W1 = const.tile([128, 8], BF)
            WV = const.tile([128, 8], BF)
            WC = const.tile([128, NSLOT * 40], BF, name="WC")
            BQK = const.tile([40, 1], F32)
            QK = const.tile([40, B, MM], F32)
            QDS = const.tile([8, B, MM], F32)
            Ss = [const.tile([16, HW], BF, name=f"S{b}") for b in range(B)]
            Rs = [const.tile([16, HW], BF, name=f"R{b}") for b in range(B)]
            VTs = [const.tile([128, NJT, 9], BF, name=f"VT{b}") for b in range(B)]
            UPQs = [const.tile([8, HW], BF, name=f"UPQ{b}") for b in range(B)]
            UPKs = [const.tile([8, HW], BF, name=f"UPK{b}") for b in range(B)]

            # ---- input DMAs (spread over the two HWDGE queues; transfers
            # serialize on the DMA farm in issue order, so XF and the conv
            # weights go first — they gate the conv critical path) ----
            nc.sync.dma_start(XF, xf[:, :, :])
            nc.scalar.dma_start(WC, wcT[:, :])
            nc.scalar.dma_start(F2, f2[:, :])
            nc.sync.dma_start(W1, w1qT[:, :])
            nc.sync.dma_start(WV, wvT[:, :])
            nc.sync.dma_start(BQK, bqk[:].rearrange("(p f) -> p f", f=1))
            for b in range(B):
                nc.sync.dma_start(Rs[b][0:8, :], emb[:, :])

            # ---- prologue compute (scratch PSUM released before main) ----
            with tc.tile_pool(name="psum_pro", bufs=1, space="PSUM") as pp:
                # strided 11x11 convs on XF with qkv-folded weights; q and k
                # share each tap matmul (q at out rows 0-7, k at 8-15), only
                # this head's 8 output channels each.
                slots = []
                for ky in range(KS):
                    for pk in range(5):
                        slots.append((ky, 2 * pk, True))
                    slots.append((ky, 10, False))
                acc = pp.tile([40, B, MM], F32, tag="acc")
                for si, (ky, kx, paired) in enumerate(slots):
                    kp = 128 if paired else 64
                    rhs = bass.AP(
                        tensor=XF.tensor,
                        offset=XF.offset + ky * PADW + kx,
                        ap=[[XF.ap[0][0], kp], [PADHW, B],
                            [STRIDE * PADW, M6], [STRIDE, M6]])
                    nc.tensor.matmul(acc, lhsT=WC[0:kp, si * 40:(si + 1) * 40],
                                     rhs=rhs,
                                     start=(si == 0), stop=(si == len(slots) - 1))
                nc.scalar.activation(QK, acc, getattr(AF, _CONV_ACT), bias=BQK)
                nc.vector.tensor_scalar_mul(QDS, QK[0:8, :, :], SCALE)

                # q_full chunks -> S rows 0-7 (b0 on DVE+ACT, b1 on Pool)
                for b in range(B):
                    for ci, (j0, nj) in enumerate(ICHUNKS):
                        pq = pp.tile([8, 512], F32, tag="pq", bufs=2)
                        nc.tensor.matmul(pq[:, :nj],
                                         lhsT=W1[b * C:(b + 1) * C, :],
                                         rhs=F2[b * C:(b + 1) * C, j0:j0 + nj],
                                         start=True, stop=True)
                        if b == 1:
                            nc.scalar.activation(Ss[b][0:8, j0:j0 + nj],
                                                 pq[:, :nj], AF.Copy)
                        elif ci < 3:
                            nc.vector.tensor_copy(Ss[b][0:8, j0:j0 + nj],
                                                  pq[:, :nj])
                        else:
                            nc.scalar.activation(Ss[b][0:8, j0:j0 + nj],
                                                 pq[:, :nj], AF.Copy)

                # v^T tiles straight from f: pt[j,c] = sum_ch f[ch,j] wv[ch,c]
                for b in range(B):
                    VT = VTs[b]
                    nc.vector.memset(VT[:, :, 0:1], 1.0)
                    ptb = pp.tile([128, NJT * 8], F32, tag="ptb", bufs=2)
                    for jt in range(NJT):
                        nc.tensor.matmul(ptb[:, jt * 8:(jt + 1) * 8],
                                         lhsT=F2[b * C:(b + 1) * C,
                                                 jt * 128:(jt + 1) * 128],
                                         rhs=WV[b * C:(b + 1) * C, :],
                                         start=True, stop=True)
                    nc.vector.tensor_copy(
                        VT[:, :, 1:9],
                        ptb.rearrange("p (a d) -> p a d", a=NJT, d=8))

                # upsampled qd/kd rows: build at partition 0 (UPQ on DVE 2x
                # copies, UPK on ACT), DMA into rows 8-15 in halves on both
                # HWDGE queues (compute engines can't write partition-start 8).
                HH = HW // 2
                for b in range(B):
                    nc.vector.tensor_copy(
                        UPQs[b].rearrange("p (a d) -> p a d", a=MM, d=64),
                        up_ap(QDS, b))
                    nc.scalar.activation(
                        UPKs[b].rearrange("p (a d) -> p a d", a=MM, d=64),
                        up_ap(QK, b, krows=True), AF.Copy)
                    nc.sync.dma_start(Ss[b][8:16, 0:HH], UPQs[b][:, 0:HH])
                    nc.scalar.dma_start(Ss[b][8:16, HH:HW], UPQs[b][:, HH:HW])
                    nc.sync.dma_start(Rs[b][8:16, 0:HH], UPKs[b][:, 0:HH])
                    nc.scalar.dma_start(Rs[b][8:16, HH:HW], UPKs[b][:, HH:HW])

            # ---- main attention loops ----
            # Flat software pipeline over (ichunk, b, jtile-pair): two
            # E-matmuls fill a 2-bank PSUM tile, ONE 1024-wide exp covers
            # both (amortizing ACT per-op overhead), and the pair's
            # O-matmuls are emitted one step later so PE never stalls on
            # the current exp. b interleaved inside ichunk so staging of
            # b=1 hides under b=0 and no drain at the b boundary.
            with tc.tile_pool(name="psum_main", bufs=1, space="PSUM") as pm:
                steps = [(b, i0, ni, jp)
                         for (i0, ni) in ICHUNKS
                         for b in range(B)
                         for jp in range(NJT // 2)]
                po_cur = [None]
                pending = [None]

                def emit_o():
                    pb_, pi0, pni, pjp, pesb = pending[0]
                    if pjp == 0:
                        po_cur[0] = pm.tile([9, 512], F32, tag="po",
                                            bufs=2, name="po")
                    po = po_cur[0]
                    VT = VTs[pb_]
                    nc.tensor.matmul(po[:, :pni], lhsT=VT[:, 2 * pjp, :],
                                     rhs=pesb[:, 0, :pni],
                                     start=(pjp == 0), stop=False)
                    nc.tensor.matmul(po[:, :pni], lhsT=VT[:, 2 * pjp + 1, :],
                                     rhs=pesb[:, 1, :pni],
                                     start=False, stop=(pjp == NJT // 2 - 1))
                    if pjp == NJT // 2 - 1:
                        rec = work.tile([1, 512], F32, tag="rec", name="rec")
                        nc.vector.reciprocal(rec[:, :pni], po[0:1, :pni])
                        pb = pm.tile([9, 512], F32, tag="po", bufs=2, name="pb")
                        nc.tensor.matmul(pb[:, :pni], lhsT=ONE9,
                                         rhs=rec[:, :pni],
                                         start=True, stop=True)
                        pbs = work.tile([9, 512], F32, tag="pbs", name="pbs")
                        nc.vector.tensor_copy(pbs[:, :pni], pb[:, :pni])
                        res = work.tile([9, 512], F32, tag="res", name="res")
                        nc.vector.tensor_mul(res[:, :pni], po[:, :pni],
                                             pbs[:, :pni])
                        nc.sync.dma_start(out[pb_, :, pi0:pi0 + pni],
                                          res[1:9, :pni])

                for step in steps:
                    b, i0, ni, jp = step
                    S, R = Ss[b], Rs[b]
                    pe2 = pm.tile([128, 2, 512], F32, tag="pe", bufs=3,
                                  name="pe2")
                    nc.tensor.matmul(pe2[:, 0, :ni],
                                     lhsT=R[:, (2 * jp) * 128:(2 * jp + 1) * 128],
                                     rhs=S[:, i0:i0 + ni],
                                     start=True, stop=True)
                    nc.tensor.matmul(pe2[:, 1, :ni],
                                     lhsT=R[:, (2 * jp + 1) * 128:(2 * jp + 2) * 128],
                                     rhs=S[:, i0:i0 + ni],
                                     start=True, stop=True)
                    esb2 = epool.tile([128, 2, 512], BF, tag="esb", bufs=6,
                                      name="esb2")
                    nc.scalar.activation(esb2[:, :, :ni], pe2[:, :, :ni], AF.Exp)
                    if pending[0] is not None:
                        emit_o()
                    pending[0] = (b, i0, ni, jp, esb2)
                emit_o()

        for _rep in range(repeat):
            _rep_body()

    if split:
        _split_waits(nc)
    return nc


def _split_waits(nc):
    """This walrus build allows at most ONE sync-wait per instruction.
    Move excess waits onto same-engine NoOps inserted just before."""
    import concourse.mybir as mybir
    ctr = 0
    for fn in nc.m.functions:
        for blk in fn.blocks:
            new = []
            for inst in blk.instructions:
                si = inst.sync_info
                waits = list(si.on_wait) if si and si.on_wait else []
                if len(waits) > 1:
                    for w in waits[:-1]:
                        ctr += 1
                        nop = mybir.InstNoOp(name=f"I-wsplit-{ctr}", ins=[], outs=[])
                        nop.engine = inst.engine
                        nop.sync_info = mybir.SyncInfo(on_wait=[w], on_update=[])
                        new.append(nop)
                    inst.sync_info = mybir.SyncInfo(
                        on_wait=[waits[-1]],
                        on_update=list(si.on_update or []))
                new.append(inst)
            blk.instructions = new


def _get_program(repeat=1):
    if repeat not in _PROGRAMS:
        _PROGRAMS[repeat] = _build_program(repeat)
    return _PROGRAMS[repeat]


def _make_in_maps(f, w_qkv, wq, bq, wk, bk, pos_h, pos_w):
    import ml_dtypes
    BF = ml_dtypes.bfloat16
    f = np.asarray(f, np.float32)
    # f in [(b,C), HW] (contiguous j) and padded+shifted XF [128, B, PADHW]
    fT = np.ascontiguousarray(f.reshape(B * C, HW))
    fpad = np.zeros((C, B, PADW, PADW), np.float32)
    fpad[:, :, PAD:PAD + H, PAD:PAD + H] = f.transpose(1, 0, 2, 3)
    xf_flat = fpad.reshape(C, B * PADHW)
    xf_shift = np.zeros_like(xf_flat)
    xf_shift[:, :-1] = xf_flat[:, 1:]
    xfv = np.concatenate([xf_flat, xf_shift], axis=0).reshape(128, B, PADHW)

    embv = np.ascontiguousarray(
        (pos_h[:, :, None] + pos_w[:, None, :]).reshape(8, HW))
    w = np.asarray(w_qkv, np.float64)[:, :, 0, 0]
    W1q, W1k = w[0:C], w[C:2 * C]       # [inner, C]
    wq = np.asarray(wq, np.float64)
    wk = np.asarray(wk, np.float64)
    # fold the 1x1 qkv conv into the 11x11 conv weights (over ic)
    wqf = np.einsum('oikl,ic->ockl', wq, W1q)   # [oc, fc, ky, kx]
    wkf = np.einsum('oikl,ic->ockl', wk, W1k)

    def pack_taps(wpq, wpk):
        # 2x [8 oc, C fc, ky, kx] -> [128, 66*40]: 5 (kx,kx+1) pairs + kx=10
        # single per ky row; partner tap weights sit at rows 64-127; q at
        # out cols 0-7 of each slot, k at 32-39 (partition-32 alignment).
        w2 = np.zeros((128, NSLOT * 40), np.float64)
        si = 0
        for ky in range(KS):
            kxs = [(2 * pk, 2 * pk + 1) for pk in range(5)] + [(10, None)]
            for kx0, kx1 in kxs:
                w2[0:64, si * 40:si * 40 + 8] = wpq[:, :, ky, kx0].T
                w2[0:64, si * 40 + 32:si * 40 + 40] = wpk[:, :, ky, kx0].T
                if kx1 is not None:
                    w2[64:128, si * 40:si * 40 + 8] = wpq[:, :, ky, kx1].T
                    w2[64:128, si * 40 + 32:si * 40 + 40] = wpk[:, :, ky, kx1].T
                si += 1
        return np.ascontiguousarray(w2).astype(BF)

    xfv = xfv.astype(BF)
    fTv = fT.astype(BF)
    embv = embv.astype(BF)
    bq = np.asarray(bq, np.float32)
    bk = np.asarray(bk, np.float32)
    in_maps = []
    for h in range(NCORES):
        hs = slice(h * 8, h * 8 + 8)
        in_maps.append({
            "xf": xfv,
            "f2": fTv,
            "w1qT": np.ascontiguousarray(
                np.tile(W1q[hs].T, (2, 1))).astype(BF),
            "wvT": np.ascontiguousarray(
                np.tile(w[2 * C:][hs].T, (2, 1))).astype(BF),
            "wcT": pack_taps(wqf[hs], wkf[hs]),
            "bqk": np.ascontiguousarray(np.concatenate(
                [bq[hs], np.zeros(24, np.float32), bk[hs]]
            ).astype(np.float32)),
            "emb": embv,
        })
    return in_maps


def _assemble(results):
    fmap = np.empty((B, C, HW), np.float32)
    for h in range(NCORES):
        fmap[:, h * 8:(h + 1) * 8, :] = results[h]["out"]
    return fmap.reshape(B, C, H, H)


def run(trace=False, **inputs):
    """Run on hardware; returns (output, BassKernelResults)."""
    from concourse.bass_utils import run_bass_kernel_spmd
    nc = _get_program()
    in_maps = _make_in_maps(**inputs)
    res = run_bass_kernel_spmd(nc, in_maps, core_ids=list(range(NCORES)),
                               trace=trace)
    return _assemble(res.results), res


def kernel(**inputs):
    out, _ = run(trace=False, **inputs)
    return out
